# revision 18
# baseline (speedup 1.0000x reference)
"""Trainium2 Bass kernel for nn_DecayTGNMemoryModule (scatter_memory).

Strategy v2 (dense event pipeline, host routing):
  - Reference semantics: for duplicate node ids only the LAST event wins and
    every event computes from the ORIGINAL memory.  Host dedupes events
    (np.unique) and splits them evenly across 8 cores; it also gathers the
    needed memory rows / messages into dense feature-major arrays (routing +
    layout only -- all arithmetic runs on device).
  - Per core, on device (all bf16 matmul/elementwise, fp32 PSUM):
      * score = exp(-decay*relu(ts-lu)) computed in a flat [nblk, BLK] layout
      * per 512-event block: replicate score across partitions with a rank-1
        matmul (ones[1,128].T @ score_row), scale the gathered memory rows,
        run the fused MLP+GRU matmuls feature-major, gate elementwise ops
        spread across ACT/DVE/Pool, stream h_new back to DRAM.
      * The message-MLP second layer is folded into the GRU input weights on
        the host: gx = relu(...) @ (W_ih@W2).T + (W_ih@b2 + b_ih)  (exact
        algebra, fewer matmuls).
  - Host writes h_new rows into a copy of the memory table (unshard) --
    no device-side scatter/gather, which was the v1 critical path (SWDGE
    descriptor generation on GpSimd: ~115us busy).
"""

import numpy as np
import ml_dtypes

import concourse.bacc as bacc
import concourse.bass as bass
import concourse.mybir as mybir
import concourse.tile as tile
from concourse.bass_utils import run_bass_kernel_spmd

NUM_NODES = 200000
MEM_DIM = 128
MSG_DIM = 172
DECAY = 0.1
N_CORES = 8
BLK = 512  # events per pipeline block

F32 = mybir.dt.float32
BF16 = mybir.dt.bfloat16
AF = mybir.ActivationFunctionType
NPBF16 = np.dtype(ml_dtypes.bfloat16)

_program_cache: dict = {}


def _build_program(C: int):
    """Build (and bacc-compile) the per-core Bass program for capacity C."""
    nblk = C // BLK
    nc = bacc.Bacc(
        "TRN2",
        target_bir_lowering=False,
        debug=False,
        enable_asserts=True,
        num_devices=N_CORES,
    )

    # per-block packed payload: [hT | msgta | msgtb(rows 0..43)] x BLK cols
    hm = nc.dram_tensor("hm", [128, 3 * C], BF16, kind="ExternalInput")
    tslu = nc.dram_tensor("tslu", [nblk, 2 * BLK], F32, kind="ExternalInput")
    wpack = nc.dram_tensor("wpack", [128, 128 + 384 + 384], BF16, kind="ExternalInput")
    w1tb = nc.dram_tensor("w1tb", [MSG_DIM - 128, 128], BF16, kind="ExternalInput")
    biases = nc.dram_tensor("biases", [128, 6], F32, kind="ExternalInput")
    ebd = nc.dram_tensor("ebd", [nblk, nblk * 128], BF16, kind="ExternalInput")
    hout = nc.dram_tensor("hout", [128, C], BF16, kind="ExternalOutput")

    with tile.TileContext(nc) as tc:
        with (
            tc.tile_pool(name="const", bufs=1) as cp,
            tc.tile_pool(name="msg", bufs=3) as mp,
            tc.tile_pool(name="wk", bufs=2) as wp,
            tc.tile_pool(name="ps", bufs=1, space="PSUM") as pp,
        ):
            # score inputs first: the rep->hs chain heads the critical path
            tslu_s = cp.tile([nblk, 2 * BLK], F32, tag="tslu")
            nc.sync.dma_start(tslu_s[:], tslu.ap())
            eb_s = cp.tile([nblk, nblk * 128], BF16, tag="eb")
            nc.sync.dma_start(eb_s[:], ebd.ap())
            wpack_s = cp.tile([128, 896], BF16, tag="wpack")
            nc.sync.dma_start(wpack_s[:], wpack.ap())
            w1tb_s = cp.tile([MSG_DIM - 128, 128], BF16, tag="w1tb")
            nc.sync.dma_start(w1tb_s[:], w1tb.ap())
            bias_s = cp.tile([128, 6], F32, tag="biases")
            nc.sync.dma_start(bias_s[:], biases.ap())

            w1ta_s = wpack_s[:, 0:128]
            wft_s = wpack_s[:, 128:512]
            whht_s = wpack_s[:, 512:896]
            bb = lambda col: bias_s[:, col : col + 1]

            # score = exp(-DECAY * max(ts - lu, 0)), flat [nblk, BLK]
            dt0 = cp.tile([nblk, BLK], F32, tag="dt0")
            nc.vector.tensor_sub(dt0[:], tslu_s[:, 0:BLK], tslu_s[:, BLK : 2 * BLK])
            dt1 = cp.tile([nblk, BLK], F32, tag="dt1")
            nc.vector.tensor_scalar_max(dt1[:], dt0[:], 0.0)
            score_s = cp.tile([nblk, BLK], BF16, tag="score")
            nc.scalar.activation(score_s[:], dt1[:], AF.Exp, scale=-DECAY)

            for b in range(nblk):
                sl = slice(b * BLK, (b + 1) * BLK)

                hmt = mp.tile([128, 3 * BLK], BF16, tag="hm")
                nc.sync.dma_start(hmt[:], hm[:, 3 * b * BLK : 3 * (b + 1) * BLK])
                hT_b = hmt[:, 0:BLK]
                ma = hmt[:, BLK : 2 * BLK]
                mb_ = hmt[: MSG_DIM - 128, 2 * BLK : 3 * BLK]

                # rep[p, e] = score[e]  (rank-1 broadcast via PE)
                rep = pp.tile([128, BLK], F32, tag="rep")
                nc.tensor.matmul(
                    rep[:], lhsT=eb_s[:, b * 128 : (b + 1) * 128], rhs=score_s[:],
                    start=True, stop=True,
                )
                hs = wp.tile([128, BLK], BF16, tag="hs")
                nc.vector.tensor_mul(hs[:], hT_b, rep[:])

                # x1 = relu(W1 @ msg + b1)   (feature-major [128f, BLK])
                px1 = pp.tile([128, BLK], F32, tag="px1")
                nc.tensor.matmul(
                    px1[:], lhsT=w1ta_s, rhs=ma, start=True, stop=False
                )
                nc.tensor.matmul(
                    px1[:], lhsT=w1tb_s[:], rhs=mb_, start=False, stop=True
                )
                x1 = wp.tile([128, BLK], BF16, tag="x1")
                nc.scalar.activation(x1[:], px1[:], AF.Relu, bias=bb(0))

                # gates: gx uses folded weights (W_ih@W2) on x1 directly
                pr = pp.tile([128, BLK], F32, tag="pr")
                nc.tensor.matmul(
                    pr[:], lhsT=wft_s[:, 0:128], rhs=x1[:], start=True, stop=False
                )
                nc.tensor.matmul(
                    pr[:], lhsT=whht_s[:, 0:128], rhs=hs[:], start=False, stop=True
                )
                pz = pp.tile([128, BLK], F32, tag="pz")
                nc.tensor.matmul(
                    pz[:], lhsT=wft_s[:, 128:256], rhs=x1[:], start=True, stop=False
                )
                nc.tensor.matmul(
                    pz[:], lhsT=whht_s[:, 128:256], rhs=hs[:], start=False, stop=True
                )
                pgx = pp.tile([128, BLK], F32, tag="pgx", bufs=2)
                nc.tensor.matmul(
                    pgx[:], lhsT=wft_s[:, 256:384], rhs=x1[:], start=True, stop=True
                )
                pgh = pp.tile([128, BLK], F32, tag="pgh", bufs=2)
                nc.tensor.matmul(
                    pgh[:], lhsT=whht_s[:, 256:384], rhs=hs[:], start=True, stop=True
                )

                r_t = wp.tile([128, BLK], BF16, tag="r")
                nc.scalar.activation(r_t[:], pr[:], AF.Sigmoid, bias=bb(2))
                z_t = wp.tile([128, BLK], BF16, tag="z")
                nc.scalar.activation(z_t[:], pz[:], AF.Sigmoid, bias=bb(3))

                # rg = (gh_n + b_hh_n) * r
                rg = wp.tile([128, BLK], BF16, tag="rg")
                nc.vector.scalar_tensor_tensor(
                    rg[:], pgh[:], bb(5), r_t[:],
                    op0=mybir.AluOpType.add, op1=mybir.AluOpType.mult,
                )
                npre = wp.tile([128, BLK], BF16, tag="npre")
                nc.vector.tensor_add(npre[:], rg[:], pgx[:])
                n_t = wp.tile([128, BLK], BF16, tag="n")
                nc.scalar.activation(n_t[:], npre[:], AF.Tanh, bias=bb(4))

                # h_new = n + z * (hs - n)
                d_t = wp.tile([128, BLK], BF16, tag="d")
                nc.gpsimd.tensor_sub(d_t[:], hs[:], n_t[:])
                zd = wp.tile([128, BLK], BF16, tag="zd")
                nc.gpsimd.tensor_mul(zd[:], z_t[:], d_t[:])
                hn = wp.tile([128, BLK], BF16, tag="hn")
                nc.vector.tensor_add(hn[:], n_t[:], zd[:])

                nc.sync.dma_start(hout[:, sl], hn[:])

    nc.compile()
    return nc


def _prepare(inputs):
    """Host routing/layout: dedupe events (last wins), dense per-core arrays."""
    node_ids = np.asarray(inputs["node_ids"])
    messages = np.asarray(inputs["messages"], dtype=np.float32)
    timestamps = np.asarray(inputs["timestamps"], dtype=np.float32)
    memory = np.asarray(inputs["memory"], dtype=np.float32)
    last_update = np.asarray(inputs["last_update"], dtype=np.float32)

    B = node_ids.shape[0]
    ids = np.clip(node_ids.astype(np.int64), 0, NUM_NODES - 1)
    uniq, rev_first = np.unique(ids[::-1], return_index=True)
    last_pos = B - 1 - rev_first  # position of last event per unique id
    n_u = uniq.shape[0]
    per = -(-n_u // N_CORES)
    C = max(BLK, -(-per // BLK) * BLK)
    nblk = C // BLK
    assert C <= 16384, f"per-core event capacity {C} too large for SBUF plan"

    # weight prep (host, O(weights)): fold W2/b2 into the GRU input weights
    W1 = np.asarray(inputs["W1"], np.float32)
    W2 = np.asarray(inputs["W2"], np.float32)
    W_ih = np.asarray(inputs["W_ih"], np.float32)
    W_hh = np.asarray(inputs["W_hh"], np.float32)
    b1 = np.asarray(inputs["b1"], np.float32)
    b2 = np.asarray(inputs["b2"], np.float32)
    b_ih = np.asarray(inputs["b_ih"], np.float32)
    b_hh = np.asarray(inputs["b_hh"], np.float32)

    Wf = W_ih @ W2  # [384, 128]
    bias_gx = W_ih @ b2 + b_ih  # [384]
    w1t = np.ascontiguousarray(W1.T)  # [172, 128]

    wpack = np.concatenate([w1t[:128], Wf.T, W_hh.T], axis=1)  # [128, 896]
    biases = np.zeros((128, 6), np.float32)
    biases[:, 0] = b1
    biases[:, 2] = bias_gx[0:128] + b_hh[0:128]
    biases[:, 3] = bias_gx[128:256] + b_hh[128:256]
    biases[:, 4] = bias_gx[256:384]
    biases[:, 5] = b_hh[256:384]
    eb = np.zeros((nblk, nblk, 128), NPBF16)
    eb[np.arange(nblk), np.arange(nblk), :] = 1.0
    wconst = {
        "wpack": wpack.astype(NPBF16),
        "w1tb": np.ascontiguousarray(w1t[128:]).astype(NPBF16),
        "biases": biases,
        "ebd": eb.reshape(nblk, nblk * 128),
    }

    in_maps = []
    chunks = []
    for c in range(N_CORES):
        lo, hi = c * per, min((c + 1) * per, n_u)
        n = max(hi - lo, 0)
        nid = uniq[lo:hi]
        pos = last_pos[lo:hi]
        chunks.append(nid)

        mT = messages[pos].T.astype(NPBF16)  # [172, n]
        # hm block b columns [3b*BLK, 3(b+1)*BLK) = [hT | msgta | msgtb]
        hmc = np.zeros((128, 3 * C), NPBF16)
        hm3 = hmc.reshape(128, nblk, 3, BLK)
        hTc = np.zeros((128, C), NPBF16)
        hTc[:, :n] = memory[nid].T.astype(NPBF16)
        hm3[:, :, 0, :] = hTc.reshape(128, nblk, BLK)
        mta = np.zeros((128, C), NPBF16)
        mta[:, :n] = mT[:128]
        hm3[:, :, 1, :] = mta.reshape(128, nblk, BLK)
        mtb = np.zeros((MSG_DIM - 128, C), NPBF16)
        mtb[:, :n] = mT[128:]
        hm3[: MSG_DIM - 128, :, 2, :] = mtb.reshape(MSG_DIM - 128, nblk, BLK)

        tslu = np.zeros((nblk, 2 * BLK), np.float32)
        ts = np.zeros(C, np.float32)
        ts[:n] = timestamps[pos]
        lu = np.zeros(C, np.float32)
        lu[:n] = last_update[nid]
        tslu[:, :BLK] = ts.reshape(nblk, BLK)
        tslu[:, BLK:] = lu.reshape(nblk, BLK)

        in_maps.append({"hm": hmc, "tslu": tslu, **wconst})
    return C, in_maps, chunks, memory


def run(inputs, trace=False, tmpdir=None):
    """Route on host, run on 8 cores, merge.  Returns (output, results)."""
    C, in_maps, chunks, memory = _prepare(inputs)
    if C not in _program_cache:
        _program_cache[C] = _build_program(C)
    nc = _program_cache[C]
    res = run_bass_kernel_spmd(
        nc, in_maps, core_ids=list(range(N_CORES)), trace=trace, tmpdir=tmpdir
    )
    out = memory.copy()
    for c in range(N_CORES):
        nid = chunks[c]
        n = nid.shape[0]
        if n:
            out[nid] = res.results[c]["hout"][:, :n].T.astype(np.float32)
    return out, res


def kernel(**inputs) -> np.ndarray:
    out, _ = run(inputs)
    return out
